# revision 1
# baseline (speedup 1.0000x reference)
"""DLRM dot-interaction + tril gather + concat kernel for Trainium2 (raw Bass).

features [B,27,128] f32, bottom_mlp_out [B,128] f32 ->
out [B, 479] = concat(bottom[b], tril(features[b] @ features[b].T)).

Data-parallel over 8 cores (B=65536 -> 8192/core). Per 128-sample megagroup:
  1. SP DMA: features chunk -> A [108, 4096] (natural layout).
  2. PE: 32 transposes (identity matmul) -> pt PSUM banks; DVE/ACT copy to
     XT [128, 3488] (d on partitions, 27 cols/sample + 32 pad cols).
  3. PE: 128 per-sample Gram matmuls (lhsT widened to 32 cols so all psum rows
     are written), col-tiled via tile_position into Gstack [128, 1024] PSUM:
     sample lb = 32*P + 16*k + b1 -> partitions [32P,32P+32), cols 512k+27b1.
  4. DVE: nested-AP copy un-interleaves Gstack -> GS2[p, 32j+c]; 32x32
     stream-transpose -> FLAT[b, 32j+i] = G_b[i,j] (sample-major).
  5. Pool: 26 strided copies gather strict lower triangle -> OUTT [128, 351].
  6. SP DMA: OUTT -> out rows [128:479]; ACT DMA ring: bottom -> out[:, 0:128]
     (DRAM->DRAM, no deps).

All synchronization is explicit (one semaphore wait per wait_ge instruction;
this walrus build rejects instructions carrying more than one embedded wait).
"""

import numpy as np

import concourse.bass as bass
import concourse.mybir as mybir

F = 27
D = 128
NPAIR = F * (F - 1) // 2  # 351
OUTW = D + NPAIR  # 479
MG = 128
N_CORES = 8
B_FULL = 65536
NB = B_FULL // N_CORES

FP32 = mybir.dt.float32


def tri(i):
    return i * (i - 1) // 2


def build_nc(nb=NB):
    assert nb % MG == 0
    n_mg = nb // MG
    nc = bass.Bass()
    feats = nc.dram_tensor("features", [nb, F, D], FP32, kind="ExternalInput")
    bottom = nc.dram_tensor("bottom_mlp_out", [nb, D], FP32, kind="ExternalInput")
    out = nc.dram_tensor("out", [nb, OUTW], FP32, kind="ExternalOutput")

    feats_flat = feats[:].rearrange("b f d -> (b f) d")  # [nb*27, 128]

    XTW = MG * F + 32  # 3488

    from contextlib import ExitStack

    with ExitStack() as ctx:
        sem = lambda n: ctx.enter_context(nc.semaphore(n))
        sb = lambda n, s: ctx.enter_context(nc.sbuf_tensor(n, s, FP32))
        ps = lambda n, s: ctx.enter_context(nc.psum_tensor(n, s, FP32))
        s_id, s_t, s_cv, s_ca, s_mm = (
            sem("s_id"), sem("s_t"), sem("s_cv"), sem("s_ca"), sem("s_mm"),
        )
        s_g2, s_tr, s_ga, s_bot, s_pad = (
            sem("s_g2"), sem("s_tr"), sem("s_ga"), sem("s_bot"), sem("s_pad"),
        )
        s_a = [sem("s_a0"), sem("s_a1")]
        s_do = [sem("s_do0"), sem("s_do1")]
        ones = sb("ones", [128, 128])
        ident = sb("ident", [128, 128])
        A = [sb("A0", [128, 3456]), sb("A1", [128, 3456])]
        XT = [sb("XT0", [128, XTW]), sb("XT1", [128, XTW])]
        GS2 = [sb("GS2_0", [128, 864]), sb("GS2_1", [128, 864])]
        FLAT = [sb("FLAT0", [128, 864]), sb("FLAT1", [128, 864])]
        OUTT = [sb("OUTT0", [128, NPAIR]), sb("OUTT1", [128, NPAIR])]
        pt = [ps(f"pt{i}", [128, 384]) for i in range(4)]
        gs = [ps("gs0", [128, 1024]), ps("gs1", [128, 1024])]
        block = ctx.enter_context(nc.Block())
        XT0, XT1 = XT

        # M1 bookkeeping: 27 transpose chunks of 128 flat-rows per megagroup,
        # grouped 3 per psum bank; banks rotate globally (sigma = 9g + R) over
        # the 4 pt tensors. Copies alternate DVE (R even, 5/mg) / ACT (R odd).
        def ndve(sigma):  # DVE copies with global index <= sigma
            if sigma < 0:
                return 0
            return 5 * (sigma // 9) + (sigma % 9 + 2) // 2

        def nact(sigma):
            if sigma < 0:
                return 0
            return 4 * (sigma // 9) + (sigma % 9 + 1) // 2

        def a_src(g):
            return feats_flat[3456 * g : 3456 * (g + 1), :].rearrange(
                "(c p) d -> p c d", p=128
            )

        @block.sync
        def _(sync):
            for g in range(n_mg):
                sl = g % 2
                if g >= 2:
                    sync.wait_ge(s_ga, g - 1)
                    sync.dma_start(
                        out[MG * (g - 2) : MG * (g - 1), D:OUTW], OUTT[g % 2][:]
                    ).then_inc(s_do[g % 2], 16)
                    sync.wait_ge(s_t, 9 * (g - 2) + 9)
                sync.dma_start(
                    A[sl][:].rearrange("p (q d) -> p q d", d=D), a_src(g)
                ).then_inc(s_a[sl], 16)
            for g in (n_mg - 2, n_mg - 1):
                if g < 0:
                    continue
                sync.wait_ge(s_ga, g + 1)
                sync.dma_start(
                    out[MG * g : MG * (g + 1), D:OUTW], OUTT[g % 2][:]
                ).then_inc(s_do[g % 2], 16)
            for sl in range(2):
                sync.wait_ge(s_do[sl], 16 * ((n_mg + 1 - sl) // 2))
            sync.wait_ge(s_bot, 16)

        @block.tensor
        def _(tensor):
            tensor.wait_ge(s_id, 2)
            tensor.wait_ge(s_pad, 2)
            for g in range(n_mg):
                sl = g % 2
                tensor.wait_ge(s_a[sl], 16 * (g // 2 + 1))
                # 27 transposes in 9 banks of 3, emitted in thirds
                for h in range(3):
                    sig = 9 * g + 3 * h - 2  # pt-bank WAR horizon
                    if ndve(sig) > 0:
                        tensor.wait_ge(s_cv, ndve(sig))
                    if nact(sig) > 0:
                        tensor.wait_ge(s_ca, nact(sig))
                    for R in range(3 * h, 3 * h + 3):
                        bank = (9 * g + R) % 4
                        for j in range(3):
                            c = 3 * R + j
                            ins = nc.tensor.transpose(
                                pt[bank][:, 128 * j : 128 * (j + 1)],
                                A[sl][:, 128 * c : 128 * (c + 1)],
                                ident[:],
                            )
                            if j == 2:
                                ins.then_inc(s_t, 1)
                # Gram matmuls, two halves of 64 samples
                if g >= 2:
                    tensor.wait_ge(s_g2, g - 1)
                for half in range(2):
                    tensor.wait_ge(s_cv, ndve(9 * g + 8) if half else ndve(9 * g + 4))
                    tensor.wait_ge(s_ca, nact(9 * g + 8) if half else nact(9 * g + 4))
                    for lb in range(64 * half, 64 * half + 64):
                        P, rem = divmod(lb, 32)
                        k, b1 = divmod(rem, 16)
                        c0 = 512 * k + F * b1
                        ins = nc.tensor.matmul(
                            gs[sl][32 * P : 32 * P + 32, c0 : c0 + F],
                            XT[sl][:, F * lb : F * lb + 32],
                            XT[sl][:, F * lb : F * lb + F],
                            start=True,
                            stop=True,
                            tile_position=(0, 32 * P),
                        )
                        if lb == 127:
                            ins.then_inc(s_mm, 1)

        @block.vector
        def _(vector):
            vector.memset(XT0[:, MG * F : XTW], 0.0).then_inc(s_pad, 1)
            vector.memset(XT1[:, MG * F : XTW], 0.0).then_inc(s_pad, 1)
            for g in range(n_mg):
                sl = g % 2
                for R in (0, 2, 4, 6, 8):
                    vector.wait_ge(s_t, 9 * g + R + 1)
                    if R == 0 and g >= 2:
                        vector.wait_ge(s_mm, g - 1)
                    vector.tensor_copy(
                        XT[sl][:, 384 * R : 384 * (R + 1)], pt[(9 * g + R) % 4][:]
                    ).then_inc(s_cv, 1)
                vector.wait_ge(s_mm, g + 1)
                in_ap = bass.AP(gs[sl], 0, [[1024, 128], [1, 27], [512, 2], [27, 16]])
                vector.tensor_copy(GS2[sl][:], in_ap).then_inc(s_g2, 1)
                if g >= 2:
                    vector.wait_ge(s_ga, g - 1)
                vector.wait_ge(s_g2, g + 1)
                vector.transpose(FLAT[sl][:], GS2[sl][:]).then_inc(s_tr, 1)

        @block.scalar
        def _(scalar):
            scalar.dma_start(out[:, 0:D], bottom[:, :]).then_inc(s_bot, 16)
            for g in range(n_mg):
                sl = g % 2
                first = True
                for R in (1, 3, 5, 7):
                    scalar.wait_ge(s_t, 9 * g + R + 1)
                    if first and g >= 2:
                        scalar.wait_ge(s_mm, g - 1)
                    first = False
                    scalar.copy(
                        XT[sl][:, 384 * R : 384 * (R + 1)], pt[(9 * g + R) % 4][:]
                    ).then_inc(s_ca, 1)

        @block.gpsimd
        def _(gpsimd):
            gpsimd.memset(ones[:], 1.0).then_inc(s_id, 1)
            gpsimd.wait_ge(s_id, 1)
            gpsimd.affine_select(
                ident[:],
                ones[:],
                pattern=[[1, 128]],
                compare_op=mybir.AluOpType.is_equal,
                fill=0.0,
                base=0,
                channel_multiplier=-1,
            ).then_inc(s_id, 1)
            for g in range(n_mg):
                sl = g % 2
                gpsimd.wait_ge(s_tr, g + 1)
                if g >= 2:
                    gpsimd.wait_ge(s_do[g % 2], 16 * (g // 2))
                for i in range(1, F):
                    src = bass.AP(FLAT[sl], i, [[864, 128], [32, i]])
                    ins = gpsimd.tensor_copy(OUTT[sl][:, tri(i) : tri(i) + i], src)
                    if i == F - 1:
                        ins.then_inc(s_ga, 1)

    return nc


_NC_CACHE = {}


def _get_nc(nb):
    if nb not in _NC_CACHE:
        _NC_CACHE[nb] = build_nc(nb)
    return _NC_CACHE[nb]


def kernel(features: np.ndarray, bottom_mlp_out: np.ndarray) -> np.ndarray:
    from concourse.bass_utils import run_bass_kernel_spmd

    B = features.shape[0]
    nb = B // N_CORES
    nc = _get_nc(nb)
    features = np.ascontiguousarray(features, dtype=np.float32)
    bottom_mlp_out = np.ascontiguousarray(bottom_mlp_out, dtype=np.float32)
    in_maps = [
        {
            "features": features[i * nb : (i + 1) * nb],
            "bottom_mlp_out": bottom_mlp_out[i * nb : (i + 1) * nb],
        }
        for i in range(N_CORES)
    ]
    res = run_bass_kernel_spmd(nc, in_maps, core_ids=list(range(N_CORES)))
    return np.concatenate([r["out"] for r in res.results], axis=0)



# revision 6
# speedup vs baseline: 1.7615x; 1.7615x over previous
"""DLRM dot-interaction + tril gather + concat kernel for Trainium2 (raw Bass).

features [B,27,128] f32, bottom_mlp_out [B,128] f32 ->
out [B, 479] = concat(bottom[b], tril(features[b] @ features[b].T)).

Data-parallel over 8 cores (B=65536 -> 8192/core). Per 128-sample megagroup g:
  1. SP DMA: features chunk -> A [128, 3456] f32 (natural layout,
     A[p, 128c+d] = X[128c+p, d]).
  2. DVE: cast A -> Abf bf16 (2x mode).  Gram runs in bf16 (tol 2e-2;
     bf16 inputs + fp32 accumulate give ~1e-3 rel err).
  3. PE: 28 bf16 transposes (27 data + 1 junk pad) via identity matmul into
     4 rotating PSUM banks pt[0..3] [128,512], 4 transposes per bank-slot
     (7 slots/mg).
  4. DVE+ACT: drain each filled bank to XT [128, 3584] bf16 (slots 1,4 on
     DVE; 0,2,3,5,6 on ACT); the PSUM fp32 values are bf16-exact so the
     cast-on-copy is lossless.  XT[d, m] = X[m, d].
  5. PE: 128 per-sample bf16 Gram matmuls (lhsT 32 cols so every psum row
     is written), col-tiled via tile_position into gs [128, 1024] PSUM:
     sample lb = 32*P + 16*k + b1 -> partitions [32P,32P+32), cols 512k+27b1.
  6. ACT: nested-AP copy un-interleaves gs -> GS2[p, 32f + s'] f32.
  7. DVE: 32x32 stream-transpose -> FLATB slot (FLAT[b, 32j+i] = G_b[i,j]),
     batched 4 megagroups per FLATB buffer.
  8. Pool: 26 strided copies per 4-mg batch gather the strict lower
     triangle -> OUTT [128, 4*351].
  9. SP DMA: OUTT -> out rows [128:479]; bottom -> out[:, 0:128] in 8
     DRAM->DRAM chunks interleaved with the feature loads.

All synchronization is explicit (one semaphore wait per wait_ge instruction;
this walrus build rejects instructions carrying more than one embedded wait).
"""

import numpy as np

import concourse.bass as bass
import concourse.mybir as mybir

F = 27
D = 128
NPAIR = F * (F - 1) // 2  # 351
OUTW = D + NPAIR  # 479
MG = 128
N_CORES = 8
B_FULL = 65536
NB = B_FULL // N_CORES

FP32 = mybir.dt.float32
BF16 = mybir.dt.bfloat16

# drain slot -> engine ('v' = DVE, 'a' = ACT) and per-engine slot rank
DVE_SLOTS = (1, 4)
ACT_SLOTS = (0, 2, 3, 5, 6)


def tri(i):
    return i * (i - 1) // 2


def drain_assign(sigma):
    """Global drain slot index -> (engine, per-engine completed-count)."""
    g, s = divmod(sigma, 7)
    if s in DVE_SLOTS:
        return "v", 2 * g + DVE_SLOTS.index(s) + 1
    return "a", 5 * g + ACT_SLOTS.index(s) + 1


def build_nc(nb=NB):
    assert nb % 512 == 0
    n_mg = nb // MG
    nb4 = n_mg // 4
    nbot = min(8, n_mg)
    bot_rows = nb // nbot
    bot_step = n_mg // nbot
    nc = bass.Bass()
    feats = nc.dram_tensor("features", [nb, F, D], FP32, kind="ExternalInput")
    bottom = nc.dram_tensor("bottom_mlp_out", [nb, D], FP32, kind="ExternalInput")
    out = nc.dram_tensor("out", [nb, OUTW], FP32, kind="ExternalOutput")

    feats_flat = feats[:].rearrange("b f d -> (b f) d")  # [nb*27, 128]

    from contextlib import ExitStack

    with ExitStack() as ctx:
        sem = lambda n: ctx.enter_context(nc.semaphore(n))
        sb = lambda n, s, dt=FP32: ctx.enter_context(nc.sbuf_tensor(n, s, dt))
        ps = lambda n, s, dt=FP32: ctx.enter_context(nc.psum_tensor(n, s, dt))
        s_a = [sem("s_a0"), sem("s_a1")]
        s_cv, s_t, s_drv, s_dra, s_mm = (
            sem("s_cv"), sem("s_t"), sem("s_drv"), sem("s_dra"), sem("s_mm"),
        )
        s_g2, s_tr, s_ga, s_do, s_bot, s_id = (
            sem("s_g2"), sem("s_tr"), sem("s_ga"), sem("s_do"), sem("s_bot"),
            sem("s_id"),
        )
        ones = sb("ones", [128, 128], BF16)
        ident = sb("ident", [128, 128], BF16)
        A = [sb("A0", [128, 3456]), sb("A1", [128, 3456])]
        Abf = [sb("Abf0", [128, 3456], BF16), sb("Abf1", [128, 3456], BF16)]
        XT = [sb("XT0", [128, 3584], BF16), sb("XT1", [128, 3584], BF16)]
        GS2 = [sb("GS2_0", [128, 864]), sb("GS2_1", [128, 864])]
        FLATB = [sb("FLATB0", [128, 3456]), sb("FLATB1", [128, 3456])]
        OUTT = [sb("OUTT0", [128, 4 * NPAIR]), sb("OUTT1", [128, 4 * NPAIR])]
        pt = [ps(f"pt{i}", [128, 512], BF16) for i in range(4)]
        gs = [ps("gs0", [128, 1024]), ps("gs1", [128, 1024])]
        block = ctx.enter_context(nc.Block())

        def a_src(g):
            return feats_flat[3456 * g : 3456 * (g + 1), :].rearrange(
                "(c p) d -> p c d", p=128
            )

        @block.sync
        def _(sync):
            def out_dma(B):
                sync.wait_ge(s_ga, B + 1)
                dst = out[512 * B : 512 * (B + 1), D:OUTW].rearrange(
                    "(m p) c -> p m c", p=128
                )
                src = OUTT[B % 2][:].rearrange("p (m c) -> p m c", c=NPAIR)
                sync.dma_start(dst, src).then_inc(s_do, 16)

            next_out = 0
            for g in range(n_mg):
                if g >= 2:
                    sync.wait_ge(s_cv, g - 1)
                sync.dma_start(
                    A[g % 2][:].rearrange("p (q d) -> p q d", d=D), a_src(g)
                ).then_inc(s_a[g % 2], 16)
                if g % bot_step == 0 and g // bot_step < nbot:
                    k = g // bot_step
                    sync.dma_start(
                        out[bot_rows * k : bot_rows * (k + 1), 0:D],
                        bottom[bot_rows * k : bot_rows * (k + 1), :],
                    ).then_inc(s_bot, 16)
                if g >= 11 and (g - 11) % 4 == 0 and next_out < nb4:
                    out_dma(next_out)
                    next_out += 1
            while next_out < nb4:
                out_dma(next_out)
                next_out += 1
            sync.wait_ge(s_do, 16 * nb4)
            sync.wait_ge(s_bot, 16 * nbot)

        @block.tensor
        def _(tensor):
            def emit_mm(h):
                tensor.wait_ge(s_drv, 2 * h + 2)
                tensor.wait_ge(s_dra, 5 * h + 5)
                if h >= 2:
                    tensor.wait_ge(s_g2, h - 1)
                for lb in range(128):
                    P, rem = divmod(lb, 32)
                    k, b1 = divmod(rem, 16)
                    c0 = 512 * k + F * b1
                    ins = tensor.matmul(
                        gs[h % 2][32 * P : 32 * P + 32, c0 : c0 + F],
                        XT[h % 2][:, F * lb : F * lb + 32],
                        XT[h % 2][:, F * lb : F * lb + F],
                        start=True,
                        stop=True,
                        tile_position=(0, 32 * P),
                    )
                    if lb == 127:
                        ins.then_inc(s_mm, 1)

            tensor.wait_ge(s_id, 2)
            for g in range(n_mg):
                tensor.wait_ge(s_cv, g + 1)
                for s in range(7):
                    sigma = 7 * g + s
                    if sigma >= 4:
                        eng, cnt = drain_assign(sigma - 4)
                        tensor.wait_ge(s_drv if eng == "v" else s_dra, cnt)
                    for u in range(4):
                        tau = 4 * s + u
                        src_c = tau if tau < F else 0
                        ins = tensor.transpose(
                            pt[sigma % 4][:, 128 * u : 128 * (u + 1)],
                            Abf[g % 2][:, 128 * src_c : 128 * (src_c + 1)],
                            ident[:],
                        )
                        if u == 3:
                            ins.then_inc(s_t, 1)
                if g >= 1:
                    emit_mm(g - 1)
            emit_mm(n_mg - 1)

        @block.vector
        def _(vector):
            def drains(h):
                for s in DVE_SLOTS:
                    sigma = 7 * h + s
                    vector.wait_ge(s_t, sigma + 1)
                    if h >= 2:
                        vector.wait_ge(s_mm, h - 1)
                    vector.tensor_copy(
                        XT[h % 2][:, 512 * s : 512 * (s + 1)], pt[sigma % 4][:]
                    ).then_inc(s_drv, 1)

            def flat_t(h):
                vector.wait_ge(s_g2, h + 1)
                if h >= 8:
                    vector.wait_ge(s_ga, h // 4 - 1)
                vector.transpose(
                    FLATB[(h // 4) % 2][:, 864 * (h % 4) : 864 * (h % 4 + 1)],
                    GS2[h % 2][:],
                ).then_inc(s_tr, 1)

            for g in range(n_mg):
                if g >= 1:
                    drains(g - 1)
                vector.wait_ge(s_a[g % 2], 16 * (g // 2 + 1))
                if g >= 2:
                    vector.wait_ge(s_t, 7 * (g - 1))
                vector.tensor_copy(Abf[g % 2][:], A[g % 2][:]).then_inc(s_cv, 1)
                if g >= 2:
                    flat_t(g - 2)
            drains(n_mg - 1)
            flat_t(n_mg - 2)
            flat_t(n_mg - 1)

        @block.scalar
        def _(scalar):
            def drains(h):
                for s in ACT_SLOTS:
                    sigma = 7 * h + s
                    scalar.wait_ge(s_t, sigma + 1)
                    if h >= 2 and s == 0:
                        scalar.wait_ge(s_mm, h - 1)
                    scalar.copy(
                        XT[h % 2][:, 512 * s : 512 * (s + 1)], pt[sigma % 4][:]
                    ).then_inc(s_dra, 1)

            def gs2(h):
                scalar.wait_ge(s_mm, h + 1)
                if h >= 2:
                    scalar.wait_ge(s_tr, h - 1)
                in_ap = bass.AP(gs[h % 2], 0, [[1024, 128], [1, 27], [512, 2], [27, 16]])
                scalar.copy(GS2[h % 2][:], in_ap).then_inc(s_g2, 1)

            for g in range(n_mg):
                if g >= 1:
                    drains(g - 1)
                if g >= 2:
                    gs2(g - 2)
            drains(n_mg - 1)
            gs2(n_mg - 2)
            gs2(n_mg - 1)

        @block.gpsimd
        def _(gpsimd):
            gpsimd.memset(ones[:], 1.0).then_inc(s_id, 1)
            gpsimd.wait_ge(s_id, 1)
            gpsimd.affine_select(
                ident[:],
                ones[:],
                pattern=[[1, 128]],
                compare_op=mybir.AluOpType.is_equal,
                fill=0.0,
                base=0,
                channel_multiplier=-1,
            ).then_inc(s_id, 1)
            for B in range(nb4):
                gpsimd.wait_ge(s_tr, 4 * B + 4)
                if B >= 2:
                    gpsimd.wait_ge(s_do, 16 * (B - 1))
                for i in range(1, F):
                    src = bass.AP(FLATB[B % 2], i, [[3456, 128], [864, 4], [32, i]])
                    dst = bass.AP(OUTT[B % 2], tri(i), [[1404, 128], [351, 4], [1, i]])
                    ins = gpsimd.tensor_copy(dst, src)
                    if i == F - 1:
                        ins.then_inc(s_ga, 1)

    return nc


_NC_CACHE = {}


def _get_nc(nb):
    if nb not in _NC_CACHE:
        _NC_CACHE[nb] = build_nc(nb)
    return _NC_CACHE[nb]


def kernel(features: np.ndarray, bottom_mlp_out: np.ndarray) -> np.ndarray:
    from concourse.bass_utils import run_bass_kernel_spmd

    B = features.shape[0]
    nb = B // N_CORES
    nc = _get_nc(nb)
    features = np.ascontiguousarray(features, dtype=np.float32)
    bottom_mlp_out = np.ascontiguousarray(bottom_mlp_out, dtype=np.float32)
    in_maps = [
        {
            "features": features[i * nb : (i + 1) * nb],
            "bottom_mlp_out": bottom_mlp_out[i * nb : (i + 1) * nb],
        }
        for i in range(N_CORES)
    ]
    res = run_bass_kernel_spmd(nc, in_maps, core_ids=list(range(N_CORES)))
    return np.concatenate([r["out"] for r in res.results], axis=0)
